# revision 1
# baseline (speedup 1.0000x reference)
"""Trainium2 Bass kernel for nn_LlamaAttention (B=1, S=2048, D=4096, H=32, KVH=8, HD=128).

Sharding (8 cores): tensor-parallel over heads. Core c owns Q heads 4c..4c+3 and
KV head c (GQA groups stay intact). Each core projects Q/K/V for its heads in a
TRANSPOSED activation layout ([head_dim, seq], head_dim on partitions), applies
RoPE via host-precomputed cos/sin tables, computes causal attention with a
transposed no-max softmax (denominators via ones-vector matmuls on the PE), then
the per-core head outputs [512, 2048] are AllGathered into the full transposed
attention output [4096, 2048]. Wo is column-parallel: core c computes output
columns [512c, 512c+512) and the host concatenates the 8 column slices.

Matmuls run in float32r (FP22 on the PE, 1 cycle/row at free-dim >= 256), which
keeps ~1e-3-level accuracy at bf16-class throughput.
"""

import math

import numpy as np

# Problem constants (hardcoded per the harness contract).
S = 2048
D = 4096
H = 32
KVH = 8
HD = 128
ROT = 64
HALF = 32
THETA = 10000.0
NCORES = 8
QH = H // NCORES  # 4 query heads per core
P = 128
CH = 512  # seq chunk (matmul moving free dim)
NCH = S // CH  # 4
DT = D // P  # 32 contraction tiles for the projections
KT = S // P  # 16 key tiles

_CACHE = {}


def _build_nc():
    import concourse.mybir as mybir
    from concourse import bacc
    from concourse.bass import ds
    from concourse.masks import make_identity
    from concourse.tile import TileContext

    f32 = mybir.dt.float32
    f32r = mybir.dt.float32r
    EXP = mybir.ActivationFunctionType.Exp

    nc = bacc.Bacc()

    xT = nc.dram_tensor("xT", [D, S], f32r, kind="ExternalInput")
    maskd = nc.dram_tensor("maskd", [KT * P, CH], f32, kind="ExternalInput")
    costab = nc.dram_tensor("costab", [ROT, S], f32, kind="ExternalInput")
    sintab = nc.dram_tensor("sintab", [ROT, S], f32, kind="ExternalInput")
    wq = nc.dram_tensor("wq", [D, QH * HD], f32r, kind="ExternalInput")
    wk = nc.dram_tensor("wk", [D, HD], f32r, kind="ExternalInput")
    wv = nc.dram_tensor("wv", [D, HD], f32r, kind="ExternalInput")
    wo = nc.dram_tensor("wo", [H * HD, CH], f32r, kind="ExternalInput")
    onesd = nc.dram_tensor("onesd", [P, 1], f32r, kind="ExternalInput")
    out = nc.dram_tensor("out", [S, CH], f32, kind="ExternalOutput")
    aout_h = [nc.dram_tensor(f"aout{h}", [HD, S], f32r) for h in range(QH)]
    aout_allh = [nc.dram_tensor(f"aout_all{h}", [NCORES * HD, S], f32r,
                                addr_space="Shared") for h in range(QH)]

    with TileContext(nc) as tc:
        with tc.tile_pool(name="ptab", bufs=1) as ptab, \
             tc.tile_pool(name="pqkv", bufs=1) as pqkv:
            costab_sb = ptab.tile([ROT, S], f32)
            nc.sync.dma_start(costab_sb[:], costab[:])
            sintab_sb = ptab.tile([ROT, S], f32)
            nc.sync.dma_start(sintab_sb[:], sintab[:])
            ones_sb = ptab.tile([P, 1], f32r)
            nc.sync.dma_start(ones_sb[:], onesd[:])
            ident_sb = ptab.tile([P, P], f32)
            make_identity(nc, ident_sb[:])

            qt_sb = pqkv.tile([P, QH, S], f32r)   # Q^T per head (roped, pre-scaled)
            kt_sb = pqkv.tile([P, S], f32r)       # K^T (roped)
            v_sb = pqkv.tile([P, KT, HD], f32r)   # V in natural [sk, hd] tiles
            aout_sb = pqkv.tile([P, QH, S], f32r)  # normalized attention out^T

            def rope(dst, src_psum, sq, prt):
                # dst[:128] <- src; then dst[0:64] = src[0:64]*cos + swap(src)[0:64]*sin_signed
                nc.scalar.copy(dst, src_psum)
                rt = prt.tile([ROT, CH], f32, tag="rt")
                nc.sync.dma_start(rt[0:HALF], dst[HALF:ROT].bitcast(f32))
                nc.sync.dma_start(rt[HALF:ROT], dst[0:HALF].bitcast(f32))
                nc.vector.tensor_mul(dst[0:ROT], dst[0:ROT], costab_sb[:, sq])
                nc.vector.tensor_mul(rt[:], rt[:], sintab_sb[:, sq])
                nc.vector.tensor_add(dst[0:ROT], dst[0:ROT], rt[:])

            # ---------------- Phase 1: QKV projections (transposed) ----------------
            with tc.tile_pool(name="pw1", bufs=1) as pw1, \
                 tc.tile_pool(name="pxt", bufs=3) as pxt, \
                 tc.tile_pool(name="pvt", bufs=2) as pvt, \
                 tc.tile_pool(name="prt", bufs=2) as prt, \
                 tc.tile_pool(name="psq", bufs=4, space="PSUM") as psq_pool, \
                 tc.tile_pool(name="psk", bufs=1, space="PSUM") as psk_pool, \
                 tc.tile_pool(name="psv", bufs=1, space="PSUM") as psv_pool, \
                 tc.tile_pool(name="pst", bufs=2, space="PSUM") as pst_pool:
                wq_sb = pw1.tile([P, DT, QH * HD], f32r)
                nc.sync.dma_start(wq_sb[:], wq.rearrange("(kt p) m -> p kt m", p=P))
                wk_sb = pw1.tile([P, DT, HD], f32r)
                nc.sync.dma_start(wk_sb[:], wk.rearrange("(kt p) m -> p kt m", p=P))
                wv_sb = pw1.tile([P, DT, HD], f32r)
                nc.sync.dma_start(wv_sb[:], wv.rearrange("(kt p) m -> p kt m", p=P))

                for c in range(NCH):
                    sq = ds(c * CH, CH)
                    psq = [psq_pool.tile([P, CH], f32, tag="psq", name=f"psq{_h}") for _h in range(QH)]
                    psk = psk_pool.tile([P, CH], f32, tag="psk")
                    psv = psv_pool.tile([P, CH], f32, tag="psv")
                    for kt in range(DT):
                        xt = pxt.tile([P, CH], f32r, tag="xt")
                        nc.sync.dma_start(xt[:], xT[ds(kt * P, P), sq])
                        xr = xt[:]
                        st = dict(start=(kt == 0), stop=(kt == DT - 1))
                        for h in range(QH):
                            nc.tensor.matmul(
                                psq[h][:], wq_sb[:, kt, ds(h * HD, HD)],
                                xr, **st)
                        nc.tensor.matmul(psk[:], wk_sb[:, kt], xr, **st)
                        nc.tensor.matmul(psv[:], wv_sb[:, kt], xr, **st)
                    for h in range(QH):
                        rope(qt_sb[:, h, sq], psq[h][:], sq, prt)
                    rope(kt_sb[:, sq], psk[:], sq, prt)
                    # V^T chunk -> natural-layout V tiles via PE transpose
                    vt = pvt.tile([P, CH], f32, tag="vt")
                    nc.scalar.copy(vt[:], psv[:])
                    for j in range(4):
                        pst = pst_pool.tile([P, P], f32, tag="pst")
                        nc.tensor.transpose(pst[:], vt[:, ds(j * P, P)], ident_sb[:])
                        nc.vector.tensor_copy(v_sb[:, 4 * c + j], pst[:])

            # ---------------- Phase 2: causal attention ----------------
            with tc.tile_pool(name="pes", bufs=6) as pes, \
                 tc.tile_pool(name="pms", bufs=16) as pms, \
                 tc.tile_pool(name="prb", bufs=2) as prb, \
                 tc.tile_pool(name="pss", bufs=3, space="PSUM") as pss_pool, \
                 tc.tile_pool(name="psd", bufs=2, space="PSUM") as psd_pool, \
                 tc.tile_pool(name="pso", bufs=2, space="PSUM") as pso_pool:
                mstiles = {}
                for t in range(KT):
                    ms = pms.tile([P, CH], f32, tag="ms", name=f"ms{t}")
                    nc.sync.dma_start(ms[:], maskd[ds(t * P, P), :])
                    mstiles[t] = ms
                for h in range(QH):
                    for c in range(NCH):
                        sq = ds(c * CH, CH)
                        ntile = 4 * c + 4
                        qr = qt_sb[:, h, sq]
                        pso = pso_pool.tile([P, CH], f32, tag="pso")
                        psd = psd_pool.tile([1, CH], f32, tag="psd")
                        for t in range(ntile):
                            pss = pss_pool.tile([P, CH], f32, tag="pss")
                            nc.tensor.matmul(
                                pss[:], kt_sb[:, ds(t * P, P)], qr,
                                start=True, stop=True)
                            es = pes.tile([P, CH], f32r, tag="es")
                            if t >= 4 * c:
                                nc.vector.tensor_add(es[:], pss[:], mstiles[t][:])
                                nc.scalar.activation(es[:], es[:], EXP)
                            else:
                                nc.scalar.activation(es[:], pss[:], EXP)
                            esr = es[:]
                            st = dict(start=(t == 0), stop=(t == ntile - 1))
                            nc.tensor.matmul(psd[:], ones_sb[:], esr, **st)
                            nc.tensor.matmul(pso[:], v_sb[:, t], esr, **st)
                        rcp = prb.tile([1, CH], f32, tag="rcp")
                        nc.vector.reciprocal(rcp[:], psd[:])
                        rb = prb.tile([P, CH], f32, tag="rb")
                        nc.gpsimd.partition_broadcast(rb[:], rcp[:])
                        nc.vector.tensor_mul(aout_sb[:, h, sq], pso[:], rb[:])
                    nc.sync.dma_start(aout_h[h][:], aout_sb[:, h, :])
                    nc.gpsimd.collective_compute(
                        "AllGather",
                        mybir.AluOpType.bypass,
                        ins=[aout_h[h][:]],
                        outs=[aout_allh[h][:]],
                        replica_groups=[list(range(NCORES))],
                    )
            # ---------------- Phase 3: output projection (column slice) ----------------
            with tc.tile_pool(name="pwo", bufs=1) as pwo, \
                 tc.tile_pool(name="pat", bufs=3) as pat, \
                 tc.tile_pool(name="pob", bufs=3) as pob, \
                 tc.tile_pool(name="psw", bufs=8, space="PSUM") as psw_pool:
                wo_sb = pwo.tile([P, DT, CH], f32r)
                wo_r = wo.rearrange("(kt p) m -> p kt m", p=P)
                for g in range(8):
                    nc.sync.dma_start(wo_sb[:, ds(g * 4, 4)], wo_r[:, ds(g * 4, 4)])
                for ss in range(S // CH):
                    psw = [psw_pool.tile([P, CH], f32, tag="psw", name=f"psw{_j}") for _j in range(4)]
                    for kt in range(DT):
                        at = pat.tile([P, CH], f32r, tag="at")
                        nc.sync.dma_start(
                            at[:], aout_allh[kt % QH][ds((kt // QH) * P, P),
                                                      ds(ss * CH, CH)])
                        wr = wo_sb[:, kt]
                        st = dict(start=(kt == 0), stop=(kt == DT - 1))
                        for j in range(4):
                            nc.tensor.matmul(
                                psw[j][:], at[:, ds(j * P, P)], wr, **st)
                    for j in range(4):
                        ob = pob.tile([P, CH], f32, tag="ob")
                        nc.vector.tensor_copy(ob[:], psw[j][:])
                        nc.sync.dma_start(out[ds(ss * CH + j * P, P), :], ob[:])

    nc.finalize()
    return nc


def _get_nc():
    if "nc" not in _CACHE:
        _CACHE["nc"] = _build_nc()
    return _CACHE["nc"]


def _host_prep(hidden_states, attention_mask, position_ids, Wq, Wk, Wv, Wo):
    hidden_states = np.asarray(hidden_states, dtype=np.float32)
    attention_mask = np.asarray(attention_mask, dtype=np.float32)
    position_ids = np.asarray(position_ids)
    Wq = np.asarray(Wq, dtype=np.float32)
    Wk = np.asarray(Wk, dtype=np.float32)
    Wv = np.asarray(Wv, dtype=np.float32)
    Wo = np.asarray(Wo, dtype=np.float32)

    x = hidden_states.reshape(S, D)
    mask = attention_mask.reshape(S, S)
    pos = position_ids.reshape(S).astype(np.float32)

    xT = np.ascontiguousarray(x.T)
    # diagonal mask tiles, transposed: maskd[128t:128(t+1)] = mask[sq-chunk, sk-tile].T
    maskd = np.concatenate(
        [np.ascontiguousarray(
            mask[(t // 4) * CH:(t // 4 + 1) * CH, t * P:(t + 1) * P].T)
         for t in range(KT)], axis=0)

    freqs = (1.0 / THETA ** (np.arange(0, HD, 2, dtype=np.float32) / HD)).astype(np.float32)
    ang = pos[:, None] * freqs[None, :]
    costab = np.ascontiguousarray(np.cos(ang).T)
    sint = np.sin(ang).T
    sintab = np.ascontiguousarray(np.concatenate([-sint[:HALF], sint[HALF:]], axis=0))

    scale = np.float32(1.0 / math.sqrt(HD))
    in_maps = []
    for c in range(NCORES):
        in_maps.append({
            "xT": xT,
            "onesd": np.ones((P, 1), np.float32),
            "maskd": maskd,
            "costab": costab,
            "sintab": sintab,
            "wq": np.ascontiguousarray(Wq[:, c * QH * HD:(c + 1) * QH * HD]) * scale,
            "wk": np.ascontiguousarray(Wk[:, c * HD:(c + 1) * HD]),
            "wv": np.ascontiguousarray(Wv[:, c * HD:(c + 1) * HD]),
            "wo": np.ascontiguousarray(Wo[:, c * CH:(c + 1) * CH]),
        })
    return in_maps


def _run(inputs, trace=False):
    from concourse.bass_utils import run_bass_kernel_spmd

    if trace:
        # NTFF profiling needs antenv.axon_hooks; provide it if the image lacks it.
        try:
            import antenv.axon_hooks  # noqa: F401
        except ImportError:
            import sys
            import types
            try:
                import trn_agent_boot.trn_boot as _tb
                _hook = _tb._ntff_profile_via_ctypes("/opt/axon/libaxon_pjrt.so")
                _m = types.ModuleType("antenv.axon_hooks")
                _m.get_axon_ntff_profile_hook = lambda: _hook
                _m.set_axon_ntff_profile_hook = lambda h: None
                sys.modules["antenv.axon_hooks"] = _m
            except Exception:
                trace = False

    nc = _get_nc()
    in_maps = _host_prep(**inputs)
    res = run_bass_kernel_spmd(nc, in_maps, core_ids=list(range(NCORES)), trace=trace)
    full = np.concatenate(
        [res.results[c]["out"] for c in range(NCORES)], axis=1)[None]
    return np.ascontiguousarray(full, dtype=np.float32), res


def kernel(hidden_states, attention_mask, position_ids, Wq, Wk, Wv, Wo):
    out, _ = _run(dict(
        hidden_states=hidden_states, attention_mask=attention_mask,
        position_ids=position_ids, Wq=Wq, Wk=Wk, Wv=Wv, Wo=Wo))
    return out



# revision 3
# speedup vs baseline: 1.3951x; 1.3951x over previous
"""Trainium2 Bass kernel for nn_LlamaAttention (B=1, S=2048, D=4096, H=32, KVH=8, HD=128).

Sharding (8 cores): tensor-parallel over heads. Core c owns Q heads 4c..4c+3 and
KV head c (GQA groups stay intact). Each core projects Q/K/V for its heads in a
TRANSPOSED activation layout ([head_dim, seq], head_dim on partitions), applies
RoPE via host-precomputed cos/sin tables, computes causal attention with a
transposed no-max softmax (denominators via ones-vector matmuls on the PE).
Per-head attention outputs are AllGathered (bf16) as soon as each head finishes;
the output projection runs h-major with SBUF accumulation so its PE work for
head-slice h depends only on AllGather h (collectives fully overlap compute).
Wo is column-parallel: core c computes output columns [512c, 512c+512) and the
host concatenates the 8 column slices.

All matmul operands are bf16 (fp32 PSUM accumulation); the causal mask is
generated on-chip with affine_select, so no mask DMA. Weight/activation DMAs are
split per contraction tile and ordered so the first matmul starts ~5us in.
"""

import math

import numpy as np

# Problem constants (hardcoded per the harness contract).
S = 2048
D = 4096
H = 32
KVH = 8
HD = 128
ROT = 64
HALF = 32
THETA = 10000.0
NCORES = 8
QH = H // NCORES  # 4 query heads per core
P = 128
CH = 512  # seq chunk (matmul moving free dim)
NCH = S // CH  # 4
DT = D // P  # 32 contraction tiles for the projections
KT = S // P  # 16 key tiles

_CACHE = {}


def _build_nc():
    import concourse.mybir as mybir
    from concourse import bacc
    from concourse.bass import ds
    from concourse.masks import make_identity
    from concourse.tile import TileContext

    f32 = mybir.dt.float32
    bf16 = mybir.dt.bfloat16
    EXP = mybir.ActivationFunctionType.Exp
    from concourse.alu_op_type import AluOpType

    nc = bacc.Bacc()

    xT = nc.dram_tensor("xT", [D, S], bf16, kind="ExternalInput")
    costab = nc.dram_tensor("costab", [ROT, S], bf16, kind="ExternalInput")
    sintab = nc.dram_tensor("sintab", [ROT, S], bf16, kind="ExternalInput")
    wq = nc.dram_tensor("wq", [D, QH * HD], bf16, kind="ExternalInput")
    wk = nc.dram_tensor("wk", [D, HD], bf16, kind="ExternalInput")
    wv = nc.dram_tensor("wv", [D, HD], bf16, kind="ExternalInput")
    wo = nc.dram_tensor("wo", [H * HD, CH], bf16, kind="ExternalInput")
    out = nc.dram_tensor("out", [S, CH], f32, kind="ExternalOutput")
    aout_h = [nc.dram_tensor(f"aout{h}", [HD, S], bf16) for h in range(QH)]
    aout_allh = [nc.dram_tensor(f"aout_all{h}", [NCORES * HD, S], bf16,
                                addr_space="Shared") for h in range(QH)]

    wq_r = wq.rearrange("(kt p) m -> p kt m", p=P)
    wk_r = wk.rearrange("(kt p) m -> p kt m", p=P)
    wv_r = wv.rearrange("(kt p) m -> p kt m", p=P)
    wo_r = wo.rearrange("(kt p) m -> p kt m", p=P)

    with TileContext(nc) as tc:
        with tc.tile_pool(name="ptab", bufs=1) as ptab, \
             tc.tile_pool(name="pqkv", bufs=1) as pqkv:
            ones_sb = ptab.tile([P, 1], bf16)
            nc.vector.memset(ones_sb[:], 1.0)
            ident_sb = ptab.tile([P, P], bf16)
            make_identity(nc, ident_sb[:])
            costab_sb = ptab.tile([ROT, S], bf16)
            sintab_sb = ptab.tile([ROT, S], bf16)
            # warm the ACT exp table set before any copy/exp traffic
            dummy = ptab.tile([1, 16], f32)
            nc.vector.memset(dummy[:], 0.0)
            nc.scalar.activation(dummy[:], dummy[:], EXP)

            qt_sb = pqkv.tile([P, QH, S], bf16)   # Q^T per head (roped, pre-scaled)
            kt_sb = pqkv.tile([P, S], bf16)       # K^T (roped)
            v_sb = pqkv.tile([P, KT, HD], bf16)   # V in natural [sk, hd] tiles
            aout_sb = pqkv.tile([P, QH, S], bf16)  # normalized attention out^T

            def rope(dst, src_psum, sq, prt):
                # dst[:128] <- src; then dst[0:64] = src[0:64]*cos + swap(src)[0:64]*sin_signed
                nc.scalar.copy(dst, src_psum)
                rt = prt.tile([ROT, CH], bf16, tag="rt")
                nc.sync.dma_start(rt[0:HALF], dst[HALF:ROT])
                nc.sync.dma_start(rt[HALF:ROT], dst[0:HALF])
                nc.vector.tensor_mul(dst[0:ROT], dst[0:ROT], costab_sb[:, sq])
                nc.vector.tensor_mul(rt[:], rt[:], sintab_sb[:, sq])
                nc.vector.tensor_add(dst[0:ROT], dst[0:ROT], rt[:])

            # ---------------- Phase 1: QKV projections (transposed) ----------------
            with tc.tile_pool(name="pw1", bufs=1) as pw1, \
                 tc.tile_pool(name="pxt", bufs=6) as pxt, \
                 tc.tile_pool(name="pvt", bufs=2) as pvt, \
                 tc.tile_pool(name="prt", bufs=4) as prt, \
                 tc.tile_pool(name="psq", bufs=4, space="PSUM") as psq_pool, \
                 tc.tile_pool(name="psk", bufs=1, space="PSUM") as psk_pool, \
                 tc.tile_pool(name="psv", bufs=1, space="PSUM") as psv_pool, \
                 tc.tile_pool(name="pst", bufs=2, space="PSUM") as pst_pool:
                wq_sb = pw1.tile([P, DT, QH * HD], bf16)
                wk_sb = pw1.tile([P, DT, HD], bf16)
                wv_sb = pw1.tile([P, DT, HD], bf16)
                xt_c0 = [None] * DT

                # critical-path-first DMA order: first kt group lands first so
                # the first matmul starts ~5us in; tables slot in early but
                # after the first few weight tiles.
                for kt in range(DT):
                    xt = pxt.tile([P, CH], bf16, tag="xt")
                    nc.sync.dma_start(xt[:], xT[ds(kt * P, P), ds(0, CH)])
                    xt_c0[kt] = xt
                    nc.sync.dma_start(wq_sb[:, kt], wq_r[:, kt])
                    nc.sync.dma_start(wk_sb[:, kt], wk_r[:, kt])
                    nc.sync.dma_start(wv_sb[:, kt], wv_r[:, kt])
                    if kt == 3:
                        nc.sync.dma_start(costab_sb[:], costab[:])
                        nc.sync.dma_start(sintab_sb[:], sintab[:])

                for c in range(NCH):
                    sq = ds(c * CH, CH)
                    psq = [psq_pool.tile([P, CH], f32, tag="psq", name=f"psq{_h}") for _h in range(QH)]
                    psk = psk_pool.tile([P, CH], f32, tag="psk")
                    psv = psv_pool.tile([P, CH], f32, tag="psv")
                    for kt in range(DT):
                        if c == 0:
                            xt = xt_c0[kt]
                        else:
                            xt = pxt.tile([P, CH], bf16, tag="xt")
                            nc.sync.dma_start(xt[:], xT[ds(kt * P, P), sq])
                        xr = xt[:]
                        st = dict(start=(kt == 0), stop=(kt == DT - 1))
                        for h in range(QH):
                            nc.tensor.matmul(
                                psq[h][:], wq_sb[:, kt, ds(h * HD, HD)],
                                xr, **st)
                        nc.tensor.matmul(psk[:], wk_sb[:, kt], xr, **st)
                        nc.tensor.matmul(psv[:], wv_sb[:, kt], xr, **st)
                    for h in range(QH):
                        rope(qt_sb[:, h, sq], psq[h][:], sq, prt)
                    rope(kt_sb[:, sq], psk[:], sq, prt)
                    # V^T chunk -> natural-layout V tiles via PE transpose
                    vt = pvt.tile([P, CH], bf16, tag="vt")
                    nc.scalar.copy(vt[:], psv[:])
                    for j in range(4):
                        pst = pst_pool.tile([P, P], bf16, tag="pst")
                        nc.tensor.transpose(pst[:], vt[:, ds(j * P, P)], ident_sb[:])
                        nc.vector.tensor_copy(v_sb[:, 4 * c + j], pst[:])

            # ---------------- Phase 2+3: attention, AllGather, output projection ----
            with tc.tile_pool(name="pwo", bufs=1) as pwo, \
                 tc.tile_pool(name="pes", bufs=8) as pes, \
                 tc.tile_pool(name="prb", bufs=3) as prb, \
                 tc.tile_pool(name="pacc", bufs=1) as pacc, \
                 tc.tile_pool(name="pat", bufs=12) as pat, \
                 tc.tile_pool(name="pob", bufs=3) as pob, \
                 tc.tile_pool(name="pss", bufs=2, space="PSUM") as pss_pool, \
                 tc.tile_pool(name="psd", bufs=1, space="PSUM") as psd_pool, \
                 tc.tile_pool(name="pso", bufs=2, space="PSUM") as pso_pool, \
                 tc.tile_pool(name="psw", bufs=2, space="PSUM") as psw_pool:
                wo_sb = pwo.tile([P, DT, CH], bf16)
                for kt in range(DT):
                    nc.sync.dma_start(wo_sb[:, kt], wo_r[:, kt])
                accum = pacc.tile([P, 4 * 4, CH], f32)

                for h in range(QH):
                    for c in range(NCH):
                        sq = ds(c * CH, CH)
                        ntile = 4 * c + 4
                        qr = qt_sb[:, h, sq]
                        pso = pso_pool.tile([P, CH], f32, tag="pso")
                        psd = psd_pool.tile([1, CH], f32, tag="psd")
                        for t in range(ntile):
                            pss = pss_pool.tile([P, CH], f32, tag="pss")
                            nc.tensor.matmul(
                                pss[:], kt_sb[:, ds(t * P, P)], qr,
                                start=True, stop=True)
                            es = pes.tile([P, CH], bf16, tag="es")
                            nc.scalar.activation(es[:], pss[:], EXP)
                            if t >= 4 * c:
                                # causal: keep es[r, q] iff 128t + r <= 512c + q
                                nc.gpsimd.affine_select(
                                    out=es[:], in_=es[:],
                                    compare_op=AluOpType.is_ge,
                                    fill=0.0,
                                    base=CH * c - P * t,
                                    pattern=[[1, CH]],
                                    channel_multiplier=-1,
                                )
                            esr = es[:]
                            st = dict(start=(t == 0), stop=(t == ntile - 1))
                            nc.tensor.matmul(psd[:], ones_sb[:], esr, **st)
                            nc.tensor.matmul(pso[:], v_sb[:, t], esr, **st)
                        rcp = prb.tile([1, CH], f32, tag="rcp")
                        nc.vector.reciprocal(rcp[:], psd[:])
                        rb = prb.tile([P, CH], f32, tag="rb")
                        nc.gpsimd.partition_broadcast(rb[:], rcp[:])
                        nc.vector.tensor_mul(aout_sb[:, h, sq], pso[:], rb[:])
                    nc.sync.dma_start(aout_h[h][:], aout_sb[:, h, :])
                    nc.gpsimd.collective_compute(
                        "AllGather",
                        mybir.AluOpType.bypass,
                        ins=[aout_h[h][:]],
                        outs=[aout_allh[h][:]],
                        replica_groups=[list(range(NCORES))],
                    )

                # Output projection, h-major: head-slice h only needs AllGather h,
                # so this work queues behind phase 2 on the PE with no stall.
                for h in range(QH):
                    for ss in range(NCH):
                        ats = []
                        for r in range(NCORES):
                            at = pat.tile([P, CH], bf16, tag="at")
                            nc.sync.dma_start(
                                at[:], aout_allh[h][ds(r * P, P), ds(ss * CH, CH)])
                            ats.append(at)
                        for j in range(4):
                            psw = psw_pool.tile([P, CH], f32, tag="psw")
                            for r in range(NCORES):
                                nc.tensor.matmul(
                                    psw[:], ats[r][:, ds(j * P, P)],
                                    wo_sb[:, 4 * r + h],
                                    start=(r == 0), stop=(r == NCORES - 1))
                            idx = ss * 4 + j
                            if h == 0:
                                nc.vector.tensor_copy(accum[:, idx], psw[:])
                            elif h < QH - 1:
                                nc.vector.tensor_add(accum[:, idx], accum[:, idx], psw[:])
                            else:
                                ob = pob.tile([P, CH], f32, tag="ob")
                                nc.vector.tensor_add(ob[:], accum[:, idx], psw[:])
                                nc.sync.dma_start(
                                    out[ds(ss * CH + j * P, P), :], ob[:])

    nc.finalize()
    return nc


def _get_nc():
    if "nc" not in _CACHE:
        _CACHE["nc"] = _build_nc()
    return _CACHE["nc"]


def _host_prep(hidden_states, attention_mask, position_ids, Wq, Wk, Wv, Wo):
    import ml_dtypes
    bf16 = ml_dtypes.bfloat16

    hidden_states = np.asarray(hidden_states, dtype=np.float32)
    position_ids = np.asarray(position_ids)
    Wq = np.asarray(Wq, dtype=np.float32)
    Wk = np.asarray(Wk, dtype=np.float32)
    Wv = np.asarray(Wv, dtype=np.float32)
    Wo = np.asarray(Wo, dtype=np.float32)

    x = hidden_states.reshape(S, D)
    pos = position_ids.reshape(S).astype(np.float32)

    xT = np.ascontiguousarray(x.T).astype(bf16)

    freqs = (1.0 / THETA ** (np.arange(0, HD, 2, dtype=np.float32) / HD)).astype(np.float32)
    ang = pos[:, None] * freqs[None, :]
    costab = np.ascontiguousarray(np.cos(ang).T).astype(bf16)
    sint = np.sin(ang).T
    sintab = np.ascontiguousarray(
        np.concatenate([-sint[:HALF], sint[HALF:]], axis=0)).astype(bf16)

    scale = np.float32(1.0 / math.sqrt(HD))
    in_maps = []
    for c in range(NCORES):
        in_maps.append({
            "xT": xT,
            "costab": costab,
            "sintab": sintab,
            "wq": (np.ascontiguousarray(Wq[:, c * QH * HD:(c + 1) * QH * HD]) * scale).astype(bf16),
            "wk": np.ascontiguousarray(Wk[:, c * HD:(c + 1) * HD]).astype(bf16),
            "wv": np.ascontiguousarray(Wv[:, c * HD:(c + 1) * HD]).astype(bf16),
            "wo": np.ascontiguousarray(Wo[:, c * CH:(c + 1) * CH]).astype(bf16),
        })
    return in_maps


def _run(inputs, trace=False):
    from concourse.bass_utils import run_bass_kernel_spmd

    if trace:
        # NTFF profiling needs antenv.axon_hooks; provide it if the image lacks it.
        try:
            import antenv.axon_hooks  # noqa: F401
        except ImportError:
            import sys
            import types
            try:
                import trn_agent_boot.trn_boot as _tb
                _hook = _tb._ntff_profile_via_ctypes("/opt/axon/libaxon_pjrt.so")
                _m = types.ModuleType("antenv.axon_hooks")
                _m.get_axon_ntff_profile_hook = lambda: _hook
                _m.set_axon_ntff_profile_hook = lambda h: None
                sys.modules["antenv.axon_hooks"] = _m
            except Exception:
                trace = False

    nc = _get_nc()
    in_maps = _host_prep(**inputs)
    res = run_bass_kernel_spmd(nc, in_maps, core_ids=list(range(NCORES)), trace=trace)
    full = np.concatenate(
        [res.results[c]["out"] for c in range(NCORES)], axis=1)[None]
    return np.ascontiguousarray(full, dtype=np.float32), res


def kernel(hidden_states, attention_mask, position_ids, Wq, Wk, Wv, Wo):
    out, _ = _run(dict(
        hidden_states=hidden_states, attention_mask=attention_mask,
        position_ids=position_ids, Wq=Wq, Wk=Wk, Wv=Wv, Wo=Wo))
    return out


# revision 6
# speedup vs baseline: 1.6374x; 1.1737x over previous
"""Trainium2 Bass kernel for nn_LlamaAttention (B=1, S=2048, D=4096, H=32, KVH=8, HD=128).

Sharding (8 cores): tensor-parallel over heads. Core c owns Q heads 4c..4c+3 and
KV head c (GQA groups stay intact). Each core projects Q/K/V for its heads in a
TRANSPOSED activation layout ([head_dim, seq], head_dim on partitions), applies
RoPE via host-precomputed cos/sin tables, computes causal attention with a
transposed no-max softmax. Per-head attention outputs are AllGathered (bf16) as
soon as each head finishes; the output projection runs h-major with SBUF
accumulation so its PE work for head-slice h depends only on AllGather h.
Wo is column-parallel: core c computes output columns [512c, 512c+512) and the
host concatenates the 8 column slices.

Attention blocks are two-pass so the PE never waits on the exp chain:
pass 1 issues all score matmuls (exp'd in pairs on ACT, causal-masked by a bf16
binary-mask multiply on DVE, row-summed into an SBUF accumulator on DVE);
pass 2 issues all attn@V matmuls plus two denominator matmuls back-to-back with
every operand already resident. All matmul operands are bf16 (fp32 PSUM).
"""

import math

import numpy as np

# Problem constants (hardcoded per the harness contract).
S = 2048
D = 4096
H = 32
KVH = 8
HD = 128
ROT = 64
HALF = 32
THETA = 10000.0
NCORES = 8
QH = H // NCORES  # 4 query heads per core
P = 128
CH = 512  # seq chunk (matmul moving free dim)
NCH = S // CH  # 4
DT = D // P  # 32 contraction tiles for the projections
KT = S // P  # 16 key tiles

_CACHE = {}


def _build_nc():
    import concourse.mybir as mybir
    from concourse import bacc
    from concourse.bass import ds
    from concourse.masks import make_identity
    from concourse.tile import TileContext

    f32 = mybir.dt.float32
    bf16 = mybir.dt.bfloat16
    EXP = mybir.ActivationFunctionType.Exp

    nc = bacc.Bacc()

    xT = nc.dram_tensor("xT", [D, S], bf16, kind="ExternalInput")
    maskd = nc.dram_tensor("maskd", [KT * P, CH], bf16, kind="ExternalInput")
    costab = nc.dram_tensor("costab", [ROT, S], bf16, kind="ExternalInput")
    sintab = nc.dram_tensor("sintab", [ROT, S], bf16, kind="ExternalInput")
    wq = nc.dram_tensor("wq", [D, QH * HD], bf16, kind="ExternalInput")
    wk = nc.dram_tensor("wk", [D, HD], bf16, kind="ExternalInput")
    wv = nc.dram_tensor("wv", [D, HD], bf16, kind="ExternalInput")
    wo = nc.dram_tensor("wo", [H * HD, CH], bf16, kind="ExternalInput")
    out = nc.dram_tensor("out", [S, CH], f32, kind="ExternalOutput")
    aout_h = [nc.dram_tensor(f"aout{h}", [HD, S], bf16) for h in range(QH)]
    aout_allh = [nc.dram_tensor(f"aout_all{h}", [NCORES * HD, S], bf16,
                                addr_space="Shared") for h in range(QH)]

    wq_r = wq.rearrange("(kt p) m -> p kt m", p=P)
    wk_r = wk.rearrange("(kt p) m -> p kt m", p=P)
    wv_r = wv.rearrange("(kt p) m -> p kt m", p=P)
    wo_r = wo.rearrange("(kt p) m -> p kt m", p=P)

    with TileContext(nc) as tc:
        with tc.tile_pool(name="ptab", bufs=1) as ptab, \
             tc.tile_pool(name="pqkv", bufs=1) as pqkv, \
             tc.tile_pool(name="pmask", bufs=1) as pmask, \
             tc.tile_pool(name="pes", bufs=10) as pes, \
             tc.tile_pool(name="pea", bufs=2) as pea, \
             tc.tile_pool(name="pau", bufs=2) as pau, \
             tc.tile_pool(name="prb", bufs=2) as prb:
            ones_sb = ptab.tile([P, 1], bf16)
            nc.vector.memset(ones_sb[:], 1.0)
            ident_sb = ptab.tile([P, P], bf16)
            make_identity(nc, ident_sb[:])
            costab_sb = ptab.tile([ROT, S], bf16)
            sintab_sb = ptab.tile([ROT, S], bf16)
            # warm the ACT exp table set before any copy/exp traffic
            dummy = ptab.tile([1, 16], f32)
            nc.vector.memset(dummy[:], 0.0)
            nc.scalar.activation(dummy[:], dummy[:], EXP)

            qt_sb = pqkv.tile([P, QH, S], bf16)   # Q^T per head (roped, pre-scaled)
            kt_sb = pqkv.tile([P, S], bf16)       # K^T (roped)
            v_sb = pqkv.tile([P, KT, HD], bf16)   # V in natural [sk, hd] tiles
            aout_sb = pqkv.tile([P, QH, S], bf16)  # normalized attention out^T
            mask_sb = pmask.tile([P, KT, CH], bf16)  # binary causal masks, diag tiles

            def rope(dst, src_psum, sq, prt):
                # dst[:128] <- src; then dst[0:64] = src[0:64]*cos + swap(src)[0:64]*sin_signed
                nc.scalar.copy(dst, src_psum)
                rt = prt.tile([ROT, CH], bf16, tag="rt")
                nc.sync.dma_start(rt[0:HALF], dst[HALF:ROT])
                nc.sync.dma_start(rt[HALF:ROT], dst[0:HALF])
                nc.vector.tensor_mul(dst[0:ROT], dst[0:ROT], costab_sb[:, sq])
                nc.vector.tensor_mul(rt[:], rt[:], sintab_sb[:, sq])
                nc.vector.tensor_add(dst[0:ROT], dst[0:ROT], rt[:])

            # ---------------- Phase 1: QKV projections (transposed) ----------------
            with tc.tile_pool(name="pw1", bufs=1) as pw1, \
                 tc.tile_pool(name="pxt", bufs=10) as pxt, \
                 tc.tile_pool(name="pvt", bufs=2) as pvt, \
                 tc.tile_pool(name="prt", bufs=4) as prt, \
                 tc.tile_pool(name="psq", bufs=4, space="PSUM") as psq_pool, \
                 tc.tile_pool(name="psk", bufs=1, space="PSUM") as psk_pool, \
                 tc.tile_pool(name="psv", bufs=1, space="PSUM") as psv_pool, \
                 tc.tile_pool(name="pst", bufs=2, space="PSUM") as pst_pool:
                wq_sb = pw1.tile([P, DT, QH * HD], bf16)
                wk_sb = pw1.tile([P, DT, HD], bf16)
                wv_sb = pw1.tile([P, DT, HD], bf16)
                xt_c0 = [None] * DT

                # critical-path-first DMA order: first kt group lands first so
                # the first matmul starts a few us in.
                for kt in range(DT):
                    xt = pxt.tile([P, CH], bf16, tag="xt")
                    nc.sync.dma_start(xt[:], xT[ds(kt * P, P), ds(0, CH)])
                    xt_c0[kt] = xt
                    nc.sync.dma_start(wq_sb[:, kt], wq_r[:, kt])
                    nc.sync.dma_start(wk_sb[:, kt], wk_r[:, kt])
                    nc.sync.dma_start(wv_sb[:, kt], wv_r[:, kt])
                    if kt == 3:
                        nc.sync.dma_start(costab_sb[:], costab[:])
                        nc.sync.dma_start(sintab_sb[:], sintab[:])
                for t in range(KT):
                    nc.sync.dma_start(mask_sb[:, t], maskd[ds(t * P, P), :])

                for c in range(NCH):
                    sq = ds(c * CH, CH)
                    psq = [psq_pool.tile([P, CH], f32, tag="psq", name=f"psq{_h}") for _h in range(QH)]
                    psk = psk_pool.tile([P, CH], f32, tag="psk")
                    psv = psv_pool.tile([P, CH], f32, tag="psv")
                    for kt in range(DT):
                        if c == 0:
                            xt = xt_c0[kt]
                        else:
                            xt = pxt.tile([P, CH], bf16, tag="xt")
                            nc.sync.dma_start(xt[:], xT[ds(kt * P, P), sq])
                        xr = xt[:]
                        st = dict(start=(kt == 0), stop=(kt == DT - 1))
                        for h in range(QH):
                            nc.tensor.matmul(
                                psq[h][:], wq_sb[:, kt, ds(h * HD, HD)],
                                xr, **st)
                        nc.tensor.matmul(psk[:], wk_sb[:, kt], xr, **st)
                        nc.tensor.matmul(psv[:], wv_sb[:, kt], xr, **st)
                    for h in range(QH):
                        rope(qt_sb[:, h, sq], psq[h][:], sq, prt)
                    rope(kt_sb[:, sq], psk[:], sq, prt)
                    # V^T chunk -> natural-layout V tiles via PE transpose
                    vt = pvt.tile([P, CH], bf16, tag="vt")
                    nc.scalar.copy(vt[:], psv[:])
                    for j in range(4):
                        pst = pst_pool.tile([P, P], bf16, tag="pst")
                        nc.tensor.transpose(pst[:], vt[:, ds(j * P, P)], ident_sb[:])
                        nc.vector.tensor_copy(v_sb[:, 4 * c + j], pst[:])

            # ---------------- Phase 2+3: attention, AllGather, output projection ----
            with tc.tile_pool(name="pwo", bufs=1) as pwo, \
                 tc.tile_pool(name="pacc", bufs=1) as pacc, \
                 tc.tile_pool(name="pat", bufs=32) as pat, \
                 tc.tile_pool(name="pob", bufs=3) as pob, \
                 tc.tile_pool(name="pss", bufs=2, space="PSUM") as pss_pool, \
                 tc.tile_pool(name="psd", bufs=1, space="PSUM") as psd_pool, \
                 tc.tile_pool(name="pso", bufs=1, space="PSUM") as pso_pool, \
                 tc.tile_pool(name="psw", bufs=2, space="PSUM") as psw_pool:
                wo_sb = pwo.tile([P, DT, CH], bf16)
                for kt in range(DT):
                    nc.sync.dma_start(wo_sb[:, kt], wo_r[:, kt])
                accum = pacc.tile([P, 4 * 4, CH], f32)

                for h in range(QH):
                    for c in range(NCH):
                        sq = ds(c * CH, CH)
                        ntile = 4 * c + 4
                        npair = ntile // 2
                        qr = qt_sb[:, h, sq]
                        esacc = pea.tile([P, 2, CH], bf16, tag="esacc")
                        # pass 1: scores -> exp (paired) -> causal mask -> row-sum acc
                        es_pairs = []
                        for pr in range(npair):
                            pss = pss_pool.tile([P, 2, CH], f32, tag="pss")
                            for k in range(2):
                                t = 2 * pr + k
                                nc.tensor.matmul(
                                    pss[:, k], kt_sb[:, ds(t * P, P)], qr,
                                    start=True, stop=True)
                            es = pes.tile([P, 2, CH], bf16, tag="es")
                            nc.scalar.activation(es[:], pss[:], EXP)
                            for k in range(2):
                                t = 2 * pr + k
                                if t >= 4 * c:
                                    nc.vector.tensor_mul(
                                        es[:, k], es[:, k], mask_sb[:, t])
                            if pr == 0:
                                nc.vector.tensor_copy(esacc[:], es[:])
                            else:
                                nc.vector.tensor_add(esacc[:], esacc[:], es[:])
                            es_pairs.append(es)
                        # pass 2: attn @ V, then denominators, all operands ready
                        pso = pso_pool.tile([P, CH], f32, tag="pso")
                        psd = psd_pool.tile([1, CH], f32, tag="psd")
                        for t in range(ntile):
                            nc.tensor.matmul(
                                pso[:], v_sb[:, t], es_pairs[t // 2][:, t % 2],
                                start=(t == 0), stop=(t == ntile - 1))
                        nc.tensor.matmul(psd[:], ones_sb[:], esacc[:, 0],
                                         start=True, stop=False)
                        nc.tensor.matmul(psd[:], ones_sb[:], esacc[:, 1],
                                         start=False, stop=True)
                        # evacuate pso early (ACT) so the bank frees even if the
                        # gpsimd broadcast is delayed behind a collective trigger
                        aout_u = pau.tile([P, CH], bf16, tag="aout_u")
                        nc.scalar.copy(aout_u[:], pso[:])
                        rcp = prb.tile([1, CH], f32, tag="rcp")
                        nc.vector.reciprocal(rcp[:], psd[:])
                        rb = prb.tile([P, CH], f32, tag="rb")
                        nc.gpsimd.partition_broadcast(rb[:], rcp[:])
                        nc.vector.tensor_mul(aout_sb[:, h, sq], aout_u[:], rb[:])
                    nc.sync.dma_start(aout_h[h][:], aout_sb[:, h, :])
                    nc.gpsimd.collective_compute(
                        "AllGather",
                        mybir.AluOpType.bypass,
                        ins=[aout_h[h][:]],
                        outs=[aout_allh[h][:]],
                        replica_groups=[list(range(NCORES))],
                    )

                # Output projection, h-major: head-slice h only needs AllGather h,
                # so this work queues behind phase 2 on the PE with no stall.
                for h in range(QH):
                    for ss in range(NCH):
                        ats = []
                        for r in range(NCORES):
                            at = pat.tile([P, CH], bf16, tag="at")
                            nc.sync.dma_start(
                                at[:], aout_allh[h][ds(r * P, P), ds(ss * CH, CH)])
                            ats.append(at)
                        for j in range(4):
                            psw = psw_pool.tile([P, CH], f32, tag="psw")
                            for r in range(NCORES):
                                nc.tensor.matmul(
                                    psw[:], ats[r][:, ds(j * P, P)],
                                    wo_sb[:, 4 * r + h],
                                    start=(r == 0), stop=(r == NCORES - 1))
                            idx = ss * 4 + j
                            if h == 0:
                                nc.vector.tensor_copy(accum[:, idx], psw[:])
                            elif h < QH - 1:
                                nc.vector.tensor_add(accum[:, idx], accum[:, idx], psw[:])
                            else:
                                ob = pob.tile([P, CH], f32, tag="ob")
                                nc.vector.tensor_add(ob[:], accum[:, idx], psw[:])
                                nc.sync.dma_start(
                                    out[ds(ss * CH + j * P, P), :], ob[:])

    nc.finalize()
    return nc


def _get_nc():
    if "nc" not in _CACHE:
        _CACHE["nc"] = _build_nc()
    return _CACHE["nc"]


def _host_prep(hidden_states, attention_mask, position_ids, Wq, Wk, Wv, Wo):
    import ml_dtypes
    bf16 = ml_dtypes.bfloat16

    hidden_states = np.asarray(hidden_states, dtype=np.float32)
    position_ids = np.asarray(position_ids)
    Wq = np.asarray(Wq, dtype=np.float32)
    Wk = np.asarray(Wk, dtype=np.float32)
    Wv = np.asarray(Wv, dtype=np.float32)
    Wo = np.asarray(Wo, dtype=np.float32)

    x = hidden_states.reshape(S, D)
    pos = position_ids.reshape(S).astype(np.float32)

    xT = np.ascontiguousarray(x.T).astype(bf16)

    # binary causal masks for the diagonal tiles, transposed layout:
    # maskd[128t + r, q] = 1.0 iff key 128t + r <= query 512*(t//4) + q
    r = np.arange(KT * P)[:, None]
    q = np.arange(CH)[None, :]
    maskd = ((r % P) + (r // P) * P <= (r // (4 * P)) * CH + q).astype(bf16)

    freqs = (1.0 / THETA ** (np.arange(0, HD, 2, dtype=np.float32) / HD)).astype(np.float32)
    ang = pos[:, None] * freqs[None, :]
    costab = np.ascontiguousarray(np.cos(ang).T).astype(bf16)
    sint = np.sin(ang).T
    sintab = np.ascontiguousarray(
        np.concatenate([-sint[:HALF], sint[HALF:]], axis=0)).astype(bf16)

    scale = np.float32(1.0 / math.sqrt(HD))
    in_maps = []
    for c in range(NCORES):
        in_maps.append({
            "xT": xT,
            "maskd": maskd,
            "costab": costab,
            "sintab": sintab,
            "wq": (np.ascontiguousarray(Wq[:, c * QH * HD:(c + 1) * QH * HD]) * scale).astype(bf16),
            "wk": np.ascontiguousarray(Wk[:, c * HD:(c + 1) * HD]).astype(bf16),
            "wv": np.ascontiguousarray(Wv[:, c * HD:(c + 1) * HD]).astype(bf16),
            "wo": np.ascontiguousarray(Wo[:, c * CH:(c + 1) * CH]).astype(bf16),
        })
    return in_maps


def _run(inputs, trace=False):
    from concourse.bass_utils import run_bass_kernel_spmd

    if trace:
        # NTFF profiling needs antenv.axon_hooks; provide it if the image lacks it.
        try:
            import antenv.axon_hooks  # noqa: F401
        except ImportError:
            import sys
            import types
            try:
                import trn_agent_boot.trn_boot as _tb
                _hook = _tb._ntff_profile_via_ctypes("/opt/axon/libaxon_pjrt.so")
                _m = types.ModuleType("antenv.axon_hooks")
                _m.get_axon_ntff_profile_hook = lambda: _hook
                _m.set_axon_ntff_profile_hook = lambda h: None
                sys.modules["antenv.axon_hooks"] = _m
            except Exception:
                trace = False

    nc = _get_nc()
    in_maps = _host_prep(**inputs)
    res = run_bass_kernel_spmd(nc, in_maps, core_ids=list(range(NCORES)), trace=trace)
    full = np.concatenate(
        [res.results[c]["out"] for c in range(NCORES)], axis=1)[None]
    return np.ascontiguousarray(full, dtype=np.float32), res


def kernel(hidden_states, attention_mask, position_ids, Wq, Wk, Wv, Wo):
    out, _ = _run(dict(
        hidden_states=hidden_states, attention_mask=attention_mask,
        position_ids=position_ids, Wq=Wq, Wk=Wk, Wv=Wv, Wo=Wo))
    return out


# revision 8
# speedup vs baseline: 1.6882x; 1.0310x over previous
"""Trainium2 Bass kernel for nn_LlamaAttention (B=1, S=2048, D=4096, H=32, KVH=8, HD=128).

Sharding (8 cores): tensor-parallel over heads. Core c owns Q heads 4c..4c+3 and
KV head c (GQA groups stay intact). Each core projects Q/K/V for its heads in a
TRANSPOSED activation layout ([head_dim, seq], head_dim on partitions), applies
RoPE, computes causal attention with a transposed no-max softmax. Per-head
attention outputs are AllGathered (bf16) as soon as each head finishes; the
output projection runs h-major with SBUF accumulation so its PE work for
head-slice h depends only on AllGather h. Wo is column-parallel: core c computes
output columns [512c, 512c+512) and the host concatenates the 8 column slices.

Key scheduling choices (from trace analysis):
- All bulk DMAs are coalesced (HWDGE sequencer issues cost ~0.6us each).
- RoPE's rotate-half runs as a DVE stream_shuffle: the host permutes the first
  64 rope rows of Wq/Wk (and the cos/sin tables) so each rotate partner sits
  +-16 lanes away inside a 32-lane shuffle group. The permutation cancels in
  the q.k dot products.
- Attention blocks are two-pass (all score matmuls, exp'd in pairs on ACT and
  causal-masked by a bf16 binary-mask multiply on DVE; then all attn@V matmuls)
  so the PE never waits on the exp chain.
- Softmax denominators come from a DVE row-sum accumulator (two matmuls per
  block instead of one per tile); 1/den is broadcast across partitions with a
  K=1 PE matmul so GpSimd runs nothing but the (blocking) AllGather triggers.
All matmul operands are bf16 (fp32 PSUM accumulation).
"""

import math

import numpy as np

# Problem constants (hardcoded per the harness contract).
S = 2048
D = 4096
H = 32
KVH = 8
HD = 128
ROT = 64
HALF = 32
THETA = 10000.0
NCORES = 8
QH = H // NCORES  # 4 query heads per core
P = 128
CH = 512  # seq chunk (matmul moving free dim)
NCH = S // CH  # 4
DT = D // P  # 32 contraction tiles for the projections
KT = S // P  # 16 key tiles

# rope-row permutation: partners (i, i+32) land +-16 apart in one 32-lane group
RPERM = list(range(0, 16)) + list(range(32, 48)) + list(range(16, 32)) + list(range(48, 64))
SHUF_MASK = list(range(16, 32)) + list(range(16))

_CACHE = {}


def _build_nc():
    import concourse.mybir as mybir
    from concourse import bacc
    from concourse.bass import ds
    from concourse.masks import make_identity
    from concourse.tile import TileContext

    f32 = mybir.dt.float32
    bf16 = mybir.dt.bfloat16
    EXP = mybir.ActivationFunctionType.Exp

    nc = bacc.Bacc()

    xT = nc.dram_tensor("xT", [D, S], bf16, kind="ExternalInput")
    maskd = nc.dram_tensor("maskd", [KT * P, CH], bf16, kind="ExternalInput")
    costab = nc.dram_tensor("costab", [ROT, S], bf16, kind="ExternalInput")
    sintab = nc.dram_tensor("sintab", [ROT, S], bf16, kind="ExternalInput")
    wq = nc.dram_tensor("wq", [D, QH * HD], bf16, kind="ExternalInput")
    wk = nc.dram_tensor("wk", [D, HD], bf16, kind="ExternalInput")
    wv = nc.dram_tensor("wv", [D, HD], bf16, kind="ExternalInput")
    wo = nc.dram_tensor("wo", [H * HD, CH], bf16, kind="ExternalInput")
    out = nc.dram_tensor("out", [S, CH], f32, kind="ExternalOutput")
    aout_h = [nc.dram_tensor(f"aout{h}", [HD, S], bf16) for h in range(QH)]
    aout_allh = [nc.dram_tensor(f"aout_all{h}", [NCORES * HD, S], bf16,
                                addr_space="Shared") for h in range(QH)]

    xT_r = xT.rearrange("(kt p) s -> p kt s", p=P)
    wq_r = wq.rearrange("(kt p) m -> p kt m", p=P)
    wk_r = wk.rearrange("(kt p) m -> p kt m", p=P)
    wv_r = wv.rearrange("(kt p) m -> p kt m", p=P)
    wo_r = wo.rearrange("(kt p) m -> p kt m", p=P)
    mask_r = maskd.rearrange("(t p) q -> p t q", p=P)
    ag_r = [ag.rearrange("(r p) s -> p r s", p=P) for ag in aout_allh]

    with TileContext(nc) as tc:
        with tc.tile_pool(name="ptab", bufs=1) as ptab, \
             tc.tile_pool(name="pqkv", bufs=1) as pqkv, \
             tc.tile_pool(name="pmask", bufs=1) as pmask, \
             tc.tile_pool(name="pes", bufs=10) as pes, \
             tc.tile_pool(name="pea", bufs=2) as pea, \
             tc.tile_pool(name="pau", bufs=3) as pau, \
             tc.tile_pool(name="prb", bufs=3) as prb:
            ones_sb = ptab.tile([P, 1], bf16)
            nc.vector.memset(ones_sb[:], 1.0)
            ones1 = ptab.tile([1, P], bf16)
            nc.vector.memset(ones1[:], 1.0)
            ident_sb = ptab.tile([P, P], bf16)
            make_identity(nc, ident_sb[:])
            costab_sb = ptab.tile([ROT, S], bf16)
            sintab_sb = ptab.tile([ROT, S], bf16)
            # warm the ACT exp table set before any copy/exp traffic
            dummy = ptab.tile([1, 16], f32)
            nc.vector.memset(dummy[:], 0.0)
            nc.scalar.activation(dummy[:], dummy[:], EXP)

            qt_sb = pqkv.tile([P, QH, S], bf16)   # Q^T per head (roped, pre-scaled)
            kt_sb = pqkv.tile([P, S], bf16)       # K^T (roped)
            v_sb = pqkv.tile([P, KT, HD], bf16)   # V in natural [sk, hd] tiles
            aout_sb = pqkv.tile([P, QH, S], bf16)  # normalized attention out^T
            mask_sb = pmask.tile([P, KT, CH], bf16)  # binary causal masks, diag tiles

            def rope(dst, src_psum, sq, prt):
                # dst <- src; dst[0:64] = src[0:64]*cos' + shuffle_16(src[0:64])*sin'
                nc.scalar.copy(dst, src_psum)
                rt = prt.tile([ROT, CH], bf16, tag="rt")
                nc.vector.stream_shuffle(rt[:], dst[0:ROT], SHUF_MASK)
                nc.vector.tensor_mul(dst[0:ROT], dst[0:ROT], costab_sb[:, sq])
                nc.vector.tensor_mul(rt[:], rt[:], sintab_sb[:, sq])
                nc.vector.tensor_add(dst[0:ROT], dst[0:ROT], rt[:])

            # ---------------- Phase 1: QKV projections (transposed) ----------------
            with tc.tile_pool(name="pw1", bufs=1) as pw1, \
                 tc.tile_pool(name="pxt", bufs=4) as pxt, \
                 tc.tile_pool(name="pvt", bufs=2) as pvt, \
                 tc.tile_pool(name="prt", bufs=4) as prt, \
                 tc.tile_pool(name="psq", bufs=4, space="PSUM") as psq_pool, \
                 tc.tile_pool(name="psk", bufs=1, space="PSUM") as psk_pool, \
                 tc.tile_pool(name="psv", bufs=1, space="PSUM") as psv_pool, \
                 tc.tile_pool(name="pst", bufs=2, space="PSUM") as pst_pool:
                wq_sb = pw1.tile([P, DT, QH * HD], bf16)
                wk_sb = pw1.tile([P, DT, HD], bf16)
                wv_sb = pw1.tile([P, DT, HD], bf16)
                xtq_c0 = [None] * 8

                # critical-path-first, coalesced DMAs: the first matmul needs
                # only xtq quad 0 + wq kt=0, both small and issued first.
                def xtq_dma(qd, sq):
                    xtq = pxt.tile([P, 4, CH], bf16, tag="xt")
                    nc.sync.dma_start(xtq[:], xT_r[:, ds(4 * qd, 4), sq])
                    return xtq

                xtq_c0[0] = xtq_dma(0, ds(0, CH))
                nc.sync.dma_start(wq_sb[:, 0], wq_r[:, 0])
                nc.sync.dma_start(wk_sb[:], wk_r[:])
                nc.sync.dma_start(wv_sb[:], wv_r[:])
                xtq_c0[1] = xtq_dma(1, ds(0, CH))
                nc.sync.dma_start(wq_sb[:, ds(1, 7)], wq_r[:, ds(1, 7)])
                xtq_c0[2] = xtq_dma(2, ds(0, CH))
                nc.sync.dma_start(wq_sb[:, ds(8, 8)], wq_r[:, ds(8, 8)])
                xtq_c0[3] = xtq_dma(3, ds(0, CH))
                nc.sync.dma_start(wq_sb[:, ds(16, 8)], wq_r[:, ds(16, 8)])
                xtq_c0[4] = xtq_dma(4, ds(0, CH))
                nc.sync.dma_start(wq_sb[:, ds(24, 8)], wq_r[:, ds(24, 8)])
                for qd in range(5, 8):
                    xtq_c0[qd] = xtq_dma(qd, ds(0, CH))
                nc.sync.dma_start(costab_sb[:], costab[:])
                nc.sync.dma_start(sintab_sb[:], sintab[:])
                nc.sync.dma_start(mask_sb[:], mask_r[:])

                for c in range(NCH):
                    sq = ds(c * CH, CH)
                    psq = [psq_pool.tile([P, CH], f32, tag="psq", name=f"psq{_h}") for _h in range(QH)]
                    psk = psk_pool.tile([P, CH], f32, tag="psk")
                    psv = psv_pool.tile([P, CH], f32, tag="psv")
                    for qd in range(8):
                        xtq = xtq_c0[qd] if c == 0 else xtq_dma(qd, sq)
                        for i in range(4):
                            kt = 4 * qd + i
                            xr = xtq[:, i]
                            st = dict(start=(kt == 0), stop=(kt == DT - 1))
                            for h in range(QH):
                                nc.tensor.matmul(
                                    psq[h][:], wq_sb[:, kt, ds(h * HD, HD)],
                                    xr, **st)
                            nc.tensor.matmul(psk[:], wk_sb[:, kt], xr, **st)
                            nc.tensor.matmul(psv[:], wv_sb[:, kt], xr, **st)
                    for h in range(QH):
                        rope(qt_sb[:, h, sq], psq[h][:], sq, prt)
                    rope(kt_sb[:, sq], psk[:], sq, prt)
                    # V^T chunk -> natural-layout V tiles via PE transpose
                    vt = pvt.tile([P, CH], bf16, tag="vt")
                    nc.scalar.copy(vt[:], psv[:])
                    for j in range(4):
                        pst = pst_pool.tile([P, P], bf16, tag="pst")
                        nc.tensor.transpose(pst[:], vt[:, ds(j * P, P)], ident_sb[:])
                        nc.vector.tensor_copy(v_sb[:, 4 * c + j], pst[:])

            # ---------------- Phase 2+3: attention, AllGather, output projection ----
            with tc.tile_pool(name="pwo", bufs=1) as pwo, \
                 tc.tile_pool(name="pacc", bufs=1) as pacc, \
                 tc.tile_pool(name="pat", bufs=16) as pat, \
                 tc.tile_pool(name="pob", bufs=3) as pob, \
                 tc.tile_pool(name="pss", bufs=2, space="PSUM") as pss_pool, \
                 tc.tile_pool(name="psd", bufs=1, space="PSUM") as psd_pool, \
                 tc.tile_pool(name="pso", bufs=1, space="PSUM") as pso_pool, \
                 tc.tile_pool(name="psw", bufs=2, space="PSUM") as psw_pool:
                wo_sb = pwo.tile([P, DT, CH], bf16)
                for g in range(4):
                    nc.sync.dma_start(wo_sb[:, ds(8 * g, 8)], wo_r[:, ds(8 * g, 8)])
                accum = pacc.tile([P, 4 * 4, CH], f32)

                def emit_norm(pend):
                    aout_u, rcp, h, sq = pend
                    rb = psd_pool.tile([P, CH], f32, tag="psd")
                    nc.tensor.matmul(rb[:], ones1[:], rcp[:], start=True, stop=True)
                    nc.vector.tensor_mul(aout_sb[:, h, sq], aout_u[:], rb[:])

                for h in range(QH):
                    pend = None
                    for c in range(NCH):
                        sq = ds(c * CH, CH)
                        ntile = 4 * c + 4
                        npair = ntile // 2
                        qr = qt_sb[:, h, sq]
                        esacc = pea.tile([P, 2, CH], bf16, tag="esacc")
                        # pass 1: scores -> exp (paired) -> causal mask -> row-sum acc
                        es_pairs = []
                        for pr in range(npair):
                            pss = pss_pool.tile([P, 2, CH], f32, tag="pss")
                            for k in range(2):
                                t = 2 * pr + k
                                nc.tensor.matmul(
                                    pss[:, k], kt_sb[:, ds(t * P, P)], qr,
                                    start=True, stop=True)
                            es = pes.tile([P, 2, CH], bf16, tag="es")
                            nc.scalar.activation(es[:], pss[:], EXP)
                            for k in range(2):
                                t = 2 * pr + k
                                if t >= 4 * c:
                                    nc.vector.tensor_mul(
                                        es[:, k], es[:, k], mask_sb[:, t])
                            if pr == 0:
                                nc.vector.tensor_copy(esacc[:], es[:])
                            else:
                                nc.vector.tensor_add(esacc[:], esacc[:], es[:])
                            es_pairs.append(es)
                        # deferred normalize of the previous block: by now its
                        # reciprocal is long ready, so the PE never stalls on it
                        if pend is not None:
                            emit_norm(pend)
                        # pass 2: attn @ V, then denominators, all operands ready
                        pso = pso_pool.tile([P, CH], f32, tag="pso")
                        psd = psd_pool.tile([1, CH], f32, tag="psd")
                        for t in range(ntile):
                            nc.tensor.matmul(
                                pso[:], v_sb[:, t], es_pairs[t // 2][:, t % 2],
                                start=(t == 0), stop=(t == ntile - 1))
                        nc.tensor.matmul(psd[:], ones_sb[:], esacc[:, 0],
                                         start=True, stop=False)
                        nc.tensor.matmul(psd[:], ones_sb[:], esacc[:, 1],
                                         start=False, stop=True)
                        # evacuate pso early (ACT) so the bank frees immediately
                        aout_u = pau.tile([P, CH], bf16, tag="aout_u")
                        nc.scalar.copy(aout_u[:], pso[:])
                        rcp = prb.tile([1, CH], bf16, tag="rcp")
                        with nc.allow_low_precision(reason="bf16 1/den feeds a bf16 broadcast matmul"):
                            nc.vector.reciprocal(rcp[:], psd[:])
                        pend = (aout_u, rcp, h, sq)
                    emit_norm(pend)
                    nc.sync.dma_start(aout_h[h][:], aout_sb[:, h, :])
                    nc.gpsimd.collective_compute(
                        "AllGather",
                        mybir.AluOpType.bypass,
                        ins=[aout_h[h][:]],
                        outs=[aout_allh[h][:]],
                        replica_groups=[list(range(NCORES))],
                    )

                # Output projection, h-major: head-slice h only needs AllGather h,
                # so this work queues behind phase 2 on the PE with no stall.
                for h in range(QH):
                    for ss in range(NCH):
                        ats = []
                        for rr in range(NCORES // 2):
                            at = pat.tile([P, 2, CH], bf16, tag="at")
                            nc.sync.dma_start(
                                at[:], ag_r[h][:, ds(2 * rr, 2), ds(ss * CH, CH)])
                            ats.append(at)
                        for j in range(4):
                            psw = psw_pool.tile([P, CH], f32, tag="psw")
                            for r in range(NCORES):
                                nc.tensor.matmul(
                                    psw[:], ats[r // 2][:, r % 2, ds(j * P, P)],
                                    wo_sb[:, 4 * r + h],
                                    start=(r == 0), stop=(r == NCORES - 1))
                            idx = ss * 4 + j
                            if h == 0:
                                nc.vector.tensor_copy(accum[:, idx], psw[:])
                            elif h < QH - 1:
                                nc.vector.tensor_add(accum[:, idx], accum[:, idx], psw[:])
                            else:
                                ob = pob.tile([P, CH], f32, tag="ob")
                                nc.vector.tensor_add(ob[:], accum[:, idx], psw[:])
                                nc.sync.dma_start(
                                    out[ds(ss * CH + j * P, P), :], ob[:])

    nc.finalize()
    return nc


def _get_nc():
    if "nc" not in _CACHE:
        _CACHE["nc"] = _build_nc()
    return _CACHE["nc"]


def _host_prep(hidden_states, attention_mask, position_ids, Wq, Wk, Wv, Wo):
    import ml_dtypes
    bf16 = ml_dtypes.bfloat16

    hidden_states = np.asarray(hidden_states, dtype=np.float32)
    position_ids = np.asarray(position_ids)
    Wq = np.asarray(Wq, dtype=np.float32)
    Wk = np.asarray(Wk, dtype=np.float32)
    Wv = np.asarray(Wv, dtype=np.float32)
    Wo = np.asarray(Wo, dtype=np.float32)

    x = hidden_states.reshape(S, D)
    pos = position_ids.reshape(S).astype(np.float32)

    xT = np.ascontiguousarray(x.T).astype(bf16)

    # binary causal masks for the diagonal tiles, transposed layout:
    # maskd[128t + r, q] = 1.0 iff key 128t + r <= query 512*(t//4) + q
    r = np.arange(KT * P)[:, None]
    q = np.arange(CH)[None, :]
    maskd = (r <= (r // (4 * P)) * CH + q).astype(bf16)

    freqs = (1.0 / THETA ** (np.arange(0, HD, 2, dtype=np.float32) / HD)).astype(np.float32)
    ang = pos[:, None] * freqs[None, :]
    costab = np.ascontiguousarray(np.cos(ang).T)
    sint = np.sin(ang).T
    sintab = np.concatenate([-sint[:HALF], sint[HALF:]], axis=0)
    perm = np.array(RPERM)
    costab = np.ascontiguousarray(costab[perm]).astype(bf16)
    sintab = np.ascontiguousarray(sintab[perm]).astype(bf16)

    # permute the first ROT rope rows of each q/k head to shuffle-group layout
    def permute_heads(w):
        w = w.copy()
        nh = w.shape[1] // HD
        for hh in range(nh):
            blk = w[:, hh * HD:hh * HD + ROT]
            w[:, hh * HD:hh * HD + ROT] = blk[:, perm]
        return w

    scale = np.float32(1.0 / math.sqrt(HD))
    in_maps = []
    for c in range(NCORES):
        wq_c = permute_heads(np.ascontiguousarray(Wq[:, c * QH * HD:(c + 1) * QH * HD])) * scale
        wk_c = permute_heads(np.ascontiguousarray(Wk[:, c * HD:(c + 1) * HD]))
        in_maps.append({
            "xT": xT,
            "maskd": maskd,
            "costab": costab,
            "sintab": sintab,
            "wq": wq_c.astype(bf16),
            "wk": wk_c.astype(bf16),
            "wv": np.ascontiguousarray(Wv[:, c * HD:(c + 1) * HD]).astype(bf16),
            "wo": np.ascontiguousarray(Wo[:, c * CH:(c + 1) * CH]).astype(bf16),
        })
    return in_maps


def _run(inputs, trace=False):
    from concourse.bass_utils import run_bass_kernel_spmd

    if trace:
        # NTFF profiling needs antenv.axon_hooks; provide it if the image lacks it.
        try:
            import antenv.axon_hooks  # noqa: F401
        except ImportError:
            import sys
            import types
            try:
                import trn_agent_boot.trn_boot as _tb
                _hook = _tb._ntff_profile_via_ctypes("/opt/axon/libaxon_pjrt.so")
                _m = types.ModuleType("antenv.axon_hooks")
                _m.get_axon_ntff_profile_hook = lambda: _hook
                _m.set_axon_ntff_profile_hook = lambda h: None
                sys.modules["antenv.axon_hooks"] = _m
            except Exception:
                trace = False

    nc = _get_nc()
    in_maps = _host_prep(**inputs)
    res = run_bass_kernel_spmd(nc, in_maps, core_ids=list(range(NCORES)), trace=trace)
    full = np.concatenate(
        [res.results[c]["out"] for c in range(NCORES)], axis=1)[None]
    return np.ascontiguousarray(full, dtype=np.float32), res


def kernel(hidden_states, attention_mask, position_ids, Wq, Wk, Wv, Wo):
    out, _ = _run(dict(
        hidden_states=hidden_states, attention_mask=attention_mask,
        position_ids=position_ids, Wq=Wq, Wk=Wk, Wv=Wv, Wo=Wo))
    return out


# revision 15
# speedup vs baseline: 1.6960x; 1.0046x over previous
"""Trainium2 Bass kernel for nn_LlamaAttention (B=1, S=2048, D=4096, H=32, KVH=8, HD=128).

Sharding (8 cores): tensor-parallel over heads. Core c owns Q heads 4c..4c+3 and
KV head c (GQA groups stay intact). Each core projects Q/K/V for its heads in a
TRANSPOSED activation layout ([head_dim, seq], head_dim on partitions), applies
RoPE, computes causal attention with a transposed no-max softmax. Per-head
attention outputs are AllGathered (bf16) as soon as each head finishes; the
output projection runs h-major with SBUF accumulation so its PE work for
head-slice h depends only on AllGather h. Wo is column-parallel: core c computes
output columns [512c, 512c+512) and the host concatenates the 8 column slices.

Key scheduling choices (from trace analysis):
- All bulk DMAs are coalesced (HWDGE sequencer issues cost ~0.6us each).
- RoPE's rotate-half runs as a DVE stream_shuffle: the host permutes the first
  64 rope rows of Wq/Wk (and the cos/sin tables) so each rotate partner sits
  +-16 lanes away inside a 32-lane shuffle group. The permutation cancels in
  the q.k dot products.
- Attention blocks are two-pass (all score matmuls, exp'd in pairs on ACT and
  causal-masked by a bf16 binary-mask multiply on DVE; then all attn@V matmuls)
  so the PE never waits on the exp chain.
- Softmax denominators come from a DVE row-sum accumulator (two matmuls per
  block instead of one per tile); 1/den is broadcast across partitions with a
  K=1 PE matmul so GpSimd runs nothing but the (blocking) AllGather triggers.
All matmul operands are bf16 (fp32 PSUM accumulation).
"""

import math

import numpy as np

# Problem constants (hardcoded per the harness contract).
S = 2048
D = 4096
H = 32
KVH = 8
HD = 128
ROT = 64
HALF = 32
THETA = 10000.0
NCORES = 8
QH = H // NCORES  # 4 query heads per core
P = 128
CH = 512  # seq chunk (matmul moving free dim)
NCH = S // CH  # 4
DT = D // P  # 32 contraction tiles for the projections
KT = S // P  # 16 key tiles

# rope-row permutation: partners (i, i+32) land +-16 apart in one 32-lane group
RPERM = list(range(0, 16)) + list(range(32, 48)) + list(range(16, 32)) + list(range(48, 64))
SHUF_MASK = list(range(16, 32)) + list(range(16))

_CACHE = {}


def _build_nc():
    import concourse.mybir as mybir
    from concourse import bacc
    from concourse.bass import ds
    from concourse.masks import make_identity
    from concourse.tile import TileContext

    f32 = mybir.dt.float32
    bf16 = mybir.dt.bfloat16
    EXP = mybir.ActivationFunctionType.Exp

    nc = bacc.Bacc()

    xT = nc.dram_tensor("xT", [D, S], bf16, kind="ExternalInput")
    maskd = nc.dram_tensor("maskd", [KT * P, CH], bf16, kind="ExternalInput")
    costab = nc.dram_tensor("costab", [ROT, S], bf16, kind="ExternalInput")
    sintab = nc.dram_tensor("sintab", [ROT, S], bf16, kind="ExternalInput")
    wq = nc.dram_tensor("wq", [D, QH * HD], bf16, kind="ExternalInput")
    wk = nc.dram_tensor("wk", [D, HD], bf16, kind="ExternalInput")
    wv = nc.dram_tensor("wv", [D, HD], bf16, kind="ExternalInput")
    wo = nc.dram_tensor("wo", [H * HD, CH], bf16, kind="ExternalInput")
    out = nc.dram_tensor("out", [S, CH], f32, kind="ExternalOutput")
    aout_h = [nc.dram_tensor(f"aout{h}", [HD, S], bf16) for h in range(QH)]
    aout_allh = [nc.dram_tensor(f"aout_all{h}", [NCORES * HD, S], bf16,
                                addr_space="Shared") for h in range(QH)]

    xT_r = xT.rearrange("(kt p) s -> p kt s", p=P)
    wq_r = wq.rearrange("(kt p) m -> p kt m", p=P)
    wk_r = wk.rearrange("(kt p) m -> p kt m", p=P)
    wv_r = wv.rearrange("(kt p) m -> p kt m", p=P)
    wo_r = wo.rearrange("(kt p) m -> p kt m", p=P)
    mask_r = maskd.rearrange("(t p) q -> p t q", p=P)
    ag_r = [ag.rearrange("(r p) s -> p r s", p=P) for ag in aout_allh]

    with TileContext(nc) as tc:
        with tc.tile_pool(name="ptab", bufs=1) as ptab, \
             tc.tile_pool(name="pqkv", bufs=1) as pqkv, \
             tc.tile_pool(name="pmask", bufs=1) as pmask, \
             tc.tile_pool(name="pes", bufs=9) as pes, \
             tc.tile_pool(name="pea", bufs=2) as pea, \
             tc.tile_pool(name="pau", bufs=5) as pau, \
             tc.tile_pool(name="prb", bufs=4) as prb:
            ones_sb = ptab.tile([P, 1], bf16)
            nc.vector.memset(ones_sb[:], 1.0)
            ident_sb = ptab.tile([P, P], bf16)
            make_identity(nc, ident_sb[:])
            costab_sb = ptab.tile([ROT, S], bf16)
            sintab_sb = ptab.tile([ROT, S], bf16)
            # warm the ACT exp table set before any copy/exp traffic
            dummy = ptab.tile([1, 16], f32)
            nc.vector.memset(dummy[:], 0.0)
            nc.scalar.activation(dummy[:], dummy[:], EXP)

            qt_sb = pqkv.tile([P, QH, S], bf16)   # Q^T per head (roped, pre-scaled)
            kt_sb = pqkv.tile([P, S], bf16)       # K^T (roped)
            v_sb = pqkv.tile([P, KT, HD], bf16)   # V in natural [sk, hd] tiles
            aout_sb = pqkv.tile([P, QH, S], bf16)  # normalized attention out^T
            mask_sb = pmask.tile([P, KT, CH], bf16)  # binary causal masks, diag tiles

            def rope(dst, src_psum, sq, prt):
                # dst <- src; dst[0:64] = src[0:64]*cos' + shuffle_16(src[0:64])*sin'
                nc.scalar.copy(dst, src_psum)
                rt = prt.tile([ROT, CH], bf16, tag="rt")
                nc.vector.stream_shuffle(rt[:], dst[0:ROT], SHUF_MASK)
                nc.vector.tensor_mul(dst[0:ROT], dst[0:ROT], costab_sb[:, sq])
                nc.vector.tensor_mul(rt[:], rt[:], sintab_sb[:, sq])
                nc.vector.tensor_add(dst[0:ROT], dst[0:ROT], rt[:])

            # ---------------- Phase 1: QKV projections (transposed) ----------------
            with tc.tile_pool(name="pw1", bufs=1) as pw1, \
                 tc.tile_pool(name="pxt", bufs=4) as pxt, \
                 tc.tile_pool(name="pvt", bufs=2) as pvt, \
                 tc.tile_pool(name="prt", bufs=4) as prt, \
                 tc.tile_pool(name="psq", bufs=4, space="PSUM") as psq_pool, \
                 tc.tile_pool(name="psk", bufs=1, space="PSUM") as psk_pool, \
                 tc.tile_pool(name="psv", bufs=1, space="PSUM") as psv_pool, \
                 tc.tile_pool(name="pst", bufs=2, space="PSUM") as pst_pool:
                wq_sb = pw1.tile([P, DT, QH * HD], bf16)
                wk_sb = pw1.tile([P, DT, HD], bf16)
                wv_sb = pw1.tile([P, DT, HD], bf16)
                xtq_c0 = [None] * 8

                # critical-path-first, coalesced DMAs: the first matmul needs
                # only xtq quad 0 + wq kt=0, both small and issued first.
                def xtq_dma(qd, sq):
                    xtq = pxt.tile([P, 4, CH], bf16, tag="xt")
                    nc.sync.dma_start(xtq[:], xT_r[:, ds(4 * qd, 4), sq])
                    return xtq

                xtq_c0[0] = xtq_dma(0, ds(0, CH))
                nc.sync.dma_start(wq_sb[:, 0], wq_r[:, 0])
                nc.sync.dma_start(wk_sb[:], wk_r[:])
                nc.sync.dma_start(wv_sb[:], wv_r[:])
                xtq_c0[1] = xtq_dma(1, ds(0, CH))
                nc.sync.dma_start(wq_sb[:, ds(1, 7)], wq_r[:, ds(1, 7)])
                xtq_c0[2] = xtq_dma(2, ds(0, CH))
                nc.sync.dma_start(wq_sb[:, ds(8, 8)], wq_r[:, ds(8, 8)])
                xtq_c0[3] = xtq_dma(3, ds(0, CH))
                nc.sync.dma_start(wq_sb[:, ds(16, 8)], wq_r[:, ds(16, 8)])
                xtq_c0[4] = xtq_dma(4, ds(0, CH))
                nc.sync.dma_start(wq_sb[:, ds(24, 8)], wq_r[:, ds(24, 8)])
                for qd in range(5, 8):
                    xtq_c0[qd] = xtq_dma(qd, ds(0, CH))
                nc.sync.dma_start(costab_sb[:], costab[:])
                nc.sync.dma_start(sintab_sb[:], sintab[:])
                nc.sync.dma_start(mask_sb[:], mask_r[:])

                for c in range(NCH):
                    sq = ds(c * CH, CH)
                    psq = [psq_pool.tile([P, CH], f32, tag="psq", name=f"psq{_h}") for _h in range(QH)]
                    psk = psk_pool.tile([P, CH], f32, tag="psk")
                    psv = psv_pool.tile([P, CH], f32, tag="psv")
                    for qd in range(8):
                        xtq = xtq_c0[qd] if c == 0 else xtq_dma(qd, sq)
                        for i in range(4):
                            kt = 4 * qd + i
                            xr = xtq[:, i]
                            st = dict(start=(kt == 0), stop=(kt == DT - 1))
                            for h in range(QH):
                                nc.tensor.matmul(
                                    psq[h][:], wq_sb[:, kt, ds(h * HD, HD)],
                                    xr, **st)
                            nc.tensor.matmul(psk[:], wk_sb[:, kt], xr, **st)
                            nc.tensor.matmul(psv[:], wv_sb[:, kt], xr, **st)
                    for h in range(QH):
                        rope(qt_sb[:, h, sq], psq[h][:], sq, prt)
                    rope(kt_sb[:, sq], psk[:], sq, prt)
                    # V^T chunk -> natural-layout V tiles via PE transpose
                    vt = pvt.tile([P, CH], bf16, tag="vt")
                    nc.scalar.copy(vt[:], psv[:])
                    for j in range(4):
                        pst = pst_pool.tile([P, P], bf16, tag="pst")
                        nc.tensor.transpose(pst[:], vt[:, ds(j * P, P)], ident_sb[:])
                        nc.vector.tensor_copy(v_sb[:, 4 * c + j], pst[:])

            # ---------------- Phase 2+3: attention, AllGather, output projection ----
            with tc.tile_pool(name="pwo", bufs=1) as pwo, \
                 tc.tile_pool(name="pacc", bufs=1) as pacc, \
                 tc.tile_pool(name="pat", bufs=14) as pat, \
                 tc.tile_pool(name="pob", bufs=3) as pob, \
                 tc.tile_pool(name="pss", bufs=2, space="PSUM") as pss_pool, \
                 tc.tile_pool(name="psd", bufs=1, space="PSUM") as psd_pool, \
                 tc.tile_pool(name="pso", bufs=1, space="PSUM") as pso_pool, \
                 tc.tile_pool(name="psw", bufs=2, space="PSUM") as psw_pool:
                wo_sb = pwo.tile([P, DT, CH], bf16)
                for g in range(4):
                    nc.sync.dma_start(wo_sb[:, ds(8 * g, 8)], wo_r[:, ds(8 * g, 8)])
                accum = pacc.tile([P, 4 * 4, CH], f32)

                # phase-3 work items for one head-slice: at-pair DMA loads and
                # 8-matmul j-groups, emitted either interleaved into phase-2
                # blocks (to fill ACT-paced PE bubbles) or in a straight run.
                p3_ats = {}

                def p3_emit(items, ngrp):
                    while items and ngrp > 0:
                        kind = items[0][0]
                        if kind == "ats":
                            _, hp, ss = items.pop(0)
                            ats = []
                            for rr in range(NCORES // 2):
                                at = pat.tile([P, 2, CH], bf16, tag="at")
                                nc.sync.dma_start(
                                    at[:], ag_r[hp][:, ds(2 * rr, 2), ds(ss * CH, CH)])
                                ats.append(at)
                            p3_ats[(hp, ss)] = ats
                        else:
                            _, hp, ss, j = items.pop(0)
                            ats = p3_ats[(hp, ss)]
                            psw = psw_pool.tile([P, CH], f32, tag="psw")
                            for r in range(NCORES):
                                nc.tensor.matmul(
                                    psw[:], ats[r // 2][:, r % 2, ds(j * P, P)],
                                    wo_sb[:, 4 * r + hp],
                                    start=(r == 0), stop=(r == NCORES - 1))
                            idx = ss * 4 + j
                            if hp == 0:
                                nc.vector.tensor_copy(accum[:, idx], psw[:])
                            elif hp < QH - 1:
                                nc.vector.tensor_add(accum[:, idx], accum[:, idx], psw[:])
                            else:
                                ob = pob.tile([P, CH], f32, tag="ob")
                                nc.vector.tensor_add(ob[:], accum[:, idx], psw[:])
                                nc.sync.dma_start(
                                    out[ds(ss * CH + j * P, P), :], ob[:])
                            ngrp -= 1

                def p3_items(hp):
                    items = []
                    for ss in range(NCH):
                        items.append(("ats", hp, ss))
                        for j in range(4):
                            items.append(("grp", hp, ss, j))
                    return items

                items_h0 = p3_items(0)

                for h in range(QH):
                    for c in range(NCH):
                        sq = ds(c * CH, CH)
                        ntile = 4 * c + 4
                        npair = ntile // 2
                        qr = qt_sb[:, h, sq]
                        esacc = pea.tile([P, 2, CH], bf16, tag="esacc")
                        # pass 1: scores -> exp (paired) -> causal mask -> row-sum acc
                        es_pairs = []
                        for pr in range(npair):
                            pss = pss_pool.tile([P, 2, CH], f32, tag="pss")
                            for k in range(2):
                                t = 2 * pr + k
                                nc.tensor.matmul(
                                    pss[:, k], kt_sb[:, ds(t * P, P)], qr,
                                    start=True, stop=True)
                            es = pes.tile([P, 2, CH], bf16, tag="es")
                            nc.scalar.activation(es[:], pss[:], EXP)
                            for k in range(2):
                                t = 2 * pr + k
                                if t >= 4 * c:
                                    nc.vector.tensor_mul(
                                        es[:, k], es[:, k], mask_sb[:, t])
                            if pr == 0:
                                nc.vector.tensor_copy(esacc[:], es[:])
                            else:
                                nc.vector.tensor_add(esacc[:], esacc[:], es[:])
                            es_pairs.append(es)
                        # fill the exp-paced pass-1 bubble with phase-3 work for
                        # head-slice 0 once its AllGather is safely complete
                        if h >= 2:
                            p3_emit(items_h0, 2)
                        # pass 2: attn @ V, then denominators, all operands ready
                        pso = pso_pool.tile([P, CH], f32, tag="pso")
                        psd = psd_pool.tile([1, CH], f32, tag="psd")
                        for t in range(ntile):
                            nc.tensor.matmul(
                                pso[:], v_sb[:, t], es_pairs[t // 2][:, t % 2],
                                start=(t == 0), stop=(t == ntile - 1))
                        nc.tensor.matmul(psd[:], ones_sb[:], esacc[:, 0],
                                         start=True, stop=False)
                        nc.tensor.matmul(psd[:], ones_sb[:], esacc[:, 1],
                                         start=False, stop=True)
                        # evacuate pso early (ACT) so the bank frees immediately;
                        # the normalize chain runs off the PE's critical path
                        aout_u = pau.tile([P, CH], bf16, tag="aout_u")
                        nc.scalar.copy(aout_u[:], pso[:])
                        rcp = prb.tile([1, CH], f32, tag="rcp")
                        nc.vector.reciprocal(rcp[:], psd[:])
                        rb = prb.tile([P, CH], f32, tag="rb")
                        nc.gpsimd.partition_broadcast(rb[:], rcp[:])
                        nc.vector.tensor_mul(aout_sb[:, h, sq], aout_u[:], rb[:])
                    nc.sync.dma_start(aout_h[h][:], aout_sb[:, h, :])
                    nc.gpsimd.collective_compute(
                        "AllGather",
                        mybir.AluOpType.bypass,
                        ins=[aout_h[h][:]],
                        outs=[aout_allh[h][:]],
                        replica_groups=[list(range(NCORES))],
                    )

                # Remaining output-projection work: drain head-slice 0 leftovers,
                # then slices 1..3 straight (their AllGathers land well before
                # the PE's queue reaches them).
                p3_emit(items_h0, 16)
                for hp in range(1, QH):
                    p3_emit(p3_items(hp), 16)

    nc.finalize()
    return nc


def _get_nc():
    if "nc" not in _CACHE:
        _CACHE["nc"] = _build_nc()
    return _CACHE["nc"]


def _host_prep(hidden_states, attention_mask, position_ids, Wq, Wk, Wv, Wo):
    import ml_dtypes
    bf16 = ml_dtypes.bfloat16

    hidden_states = np.asarray(hidden_states, dtype=np.float32)
    position_ids = np.asarray(position_ids)
    Wq = np.asarray(Wq, dtype=np.float32)
    Wk = np.asarray(Wk, dtype=np.float32)
    Wv = np.asarray(Wv, dtype=np.float32)
    Wo = np.asarray(Wo, dtype=np.float32)

    x = hidden_states.reshape(S, D)
    pos = position_ids.reshape(S).astype(np.float32)

    xT = np.ascontiguousarray(x.T).astype(bf16)

    # binary causal masks for the diagonal tiles, transposed layout:
    # maskd[128t + r, q] = 1.0 iff key 128t + r <= query 512*(t//4) + q
    r = np.arange(KT * P)[:, None]
    q = np.arange(CH)[None, :]
    maskd = (r <= (r // (4 * P)) * CH + q).astype(bf16)

    freqs = (1.0 / THETA ** (np.arange(0, HD, 2, dtype=np.float32) / HD)).astype(np.float32)
    ang = pos[:, None] * freqs[None, :]
    costab = np.ascontiguousarray(np.cos(ang).T)
    sint = np.sin(ang).T
    sintab = np.concatenate([-sint[:HALF], sint[HALF:]], axis=0)
    perm = np.array(RPERM)
    costab = np.ascontiguousarray(costab[perm]).astype(bf16)
    sintab = np.ascontiguousarray(sintab[perm]).astype(bf16)

    # permute the first ROT rope rows of each q/k head to shuffle-group layout
    def permute_heads(w):
        w = w.copy()
        nh = w.shape[1] // HD
        for hh in range(nh):
            blk = w[:, hh * HD:hh * HD + ROT]
            w[:, hh * HD:hh * HD + ROT] = blk[:, perm]
        return w

    scale = np.float32(1.0 / math.sqrt(HD))
    in_maps = []
    for c in range(NCORES):
        wq_c = permute_heads(np.ascontiguousarray(Wq[:, c * QH * HD:(c + 1) * QH * HD])) * scale
        wk_c = permute_heads(np.ascontiguousarray(Wk[:, c * HD:(c + 1) * HD]))
        in_maps.append({
            "xT": xT,
            "maskd": maskd,
            "costab": costab,
            "sintab": sintab,
            "wq": wq_c.astype(bf16),
            "wk": wk_c.astype(bf16),
            "wv": np.ascontiguousarray(Wv[:, c * HD:(c + 1) * HD]).astype(bf16),
            "wo": np.ascontiguousarray(Wo[:, c * CH:(c + 1) * CH]).astype(bf16),
        })
    return in_maps


def _run(inputs, trace=False):
    from concourse.bass_utils import run_bass_kernel_spmd

    if trace:
        # NTFF profiling needs antenv.axon_hooks; provide it if the image lacks it.
        try:
            import antenv.axon_hooks  # noqa: F401
        except ImportError:
            import sys
            import types
            try:
                import trn_agent_boot.trn_boot as _tb
                _hook = _tb._ntff_profile_via_ctypes("/opt/axon/libaxon_pjrt.so")
                _m = types.ModuleType("antenv.axon_hooks")
                _m.get_axon_ntff_profile_hook = lambda: _hook
                _m.set_axon_ntff_profile_hook = lambda h: None
                sys.modules["antenv.axon_hooks"] = _m
            except Exception:
                trace = False

    nc = _get_nc()
    in_maps = _host_prep(**inputs)
    res = run_bass_kernel_spmd(nc, in_maps, core_ids=list(range(NCORES)), trace=trace)
    full = np.concatenate(
        [res.results[c]["out"] for c in range(NCORES)], axis=1)[None]
    return np.ascontiguousarray(full, dtype=np.float32), res


def kernel(hidden_states, attention_mask, position_ids, Wq, Wk, Wv, Wo):
    out, _ = _run(dict(
        hidden_states=hidden_states, attention_mask=attention_mask,
        position_ids=position_ids, Wq=Wq, Wk=Wk, Wv=Wv, Wo=Wo))
    return out
